# revision 34
# baseline (speedup 1.0000x reference)
"""Causal single-head attention with per-batch length masking, on 8 trn2 cores.

Problem: x[8,2048,1024] f32, Wq/Wk/Wv[1024,64] f32, lengths[8] int64.
  q,k,v = x@W*;  s = q@k^T (causal + length-pair mask, -inf);  s *= H^-0.5
  out = softmax(s) @ v          -> [8, 2048, 64] f32

Math note: for row i < len: every causal key j<=i is also valid (j < len), so
the pair-mask never bites -> plain causal softmax. For row i >= len: only the
diagonal survives -> out[i] = v[i]. So: compute pure causal attention and
blend rows >= len with v.

Sharding: data-parallel over batch, one batch element per NeuronCore.

Per-core kernel design (v3, all-bf16, engine-balanced):
  - host passes x transposed+cast to bf16, repacked chunk-major so every DMA
    is one contiguous burst; input DMAs spread over 3 queues.
  - 8 warm-up matmuls during the DMA ramp release the PE HAM clock gate so
    real matmuls run at 2.4GHz from the start.
  - interleaved phases per 512-column quarter c: projection chunk c, then
    attention quarter c (j-tiles 0..4c+3). PE stays dense; scalar engine
    (exp-only) starts early because exp is near-critical (~22us).
  - scores computed transposed (sT[j,i]) so exp output feeds PV directly
    (v stationary, pT moving); diag causal masking via gpsimd affine_select
    on pT; denominators via two ones-columns on v (set once at startup).
  - transposes are bf16 data-stationary matmuls against an identity moving
    operand, grouped 4-at-a-time into one psum tile.
  - psum pools: PE can run 4 score-matmuls ahead of exp.
  - output written p-major ([128, 16, 64]) and unshuffled on host.
"""

import sys

import numpy as np

try:
    import concourse.bass as bass  # noqa: F401
except ImportError:
    sys.path.insert(0, "/opt/trn_rl_repo")

import concourse.bass as bass
import concourse.mybir as mybir
import concourse.tile as tile
from concourse import bacc
from concourse.bass_utils import run_bass_kernel_spmd
from concourse.masks import make_identity

F32 = mybir.dt.float32
BF16 = mybir.dt.bfloat16

B, T, E, H = 8, 2048, 1024, 64
HP = H + 2       # v augmented with 2 ones-columns (denominator trick)
P = 128          # partitions
CH = 512         # i-chunk width (quarter)
ET = E // P      # 8 e-tiles
NCH = T // CH    # 4 chunks
NIT = T // P     # 16 i-tiles
SCALE = float(H) ** -0.5
NWARM = 3        # PE warm-up matmuls during DMA ramp


def build_nc():
    nc = bacc.Bacc(
        "TRN2",
        target_bir_lowering=False,
        debug=False,
        num_devices=B,
    )

    # xt repacked on host: block (c, e) of [128, 512] is contiguous
    xt_d = nc.dram_tensor("xt", [NCH * ET * P, CH], BF16, kind="ExternalInput").ap()
    # weights host-packed p-major: [128, e*F]
    wqk_d = nc.dram_tensor("wqk", [P, ET * 2 * H], BF16, kind="ExternalInput").ap()
    wv_d = nc.dram_tensor("wv", [P, ET * H], BF16, kind="ExternalInput").ap()
    m_d = nc.dram_tensor("m", [P, NIT], F32, kind="ExternalInput").ap()
    im_d = nc.dram_tensor("im", [P, NIT], F32, kind="ExternalInput").ap()
    nb_d = nc.dram_tensor("nb", [1, 1], mybir.dt.uint32, kind="ExternalInput").ap()
    # output p-major [128, 16*64], host unshuffles
    out_d = nc.dram_tensor("out", [P, NIT * H], F32, kind="ExternalOutput").ap()

    with tile.TileContext(nc) as tc:
        with (
            tc.tile_pool(name="const", bufs=1) as cpool,
            tc.tile_pool(name="xt", bufs=1) as xtpool,
            tc.tile_pool(name="qk", bufs=1) as qkpool,
            tc.tile_pool(name="pt", bufs=6) as ptpool,
            tc.tile_pool(name="blend", bufs=4) as blpool,
            tc.tile_pool(name="ob", bufs=2) as obpool,
            tc.tile_pool(name="pp", bufs=2, space="PSUM") as pp,
            tc.tile_pool(name="ps_t", bufs=1, space="PSUM") as ps_t,
            tc.tile_pool(name="ps_s", bufs=2, space="PSUM") as ps_s,
            tc.tile_pool(name="ps_po", bufs=1, space="PSUM") as ps_po,
        ):
            # ---- constants ----
            ident = cpool.tile([HP, HP], F32, tag="ident")
            make_identity(nc, ident[:])
            ident_b = cpool.tile([HP, HP], BF16, tag="ident_b")
            nc.vector.tensor_copy(ident_b[:], ident[:])
            warm = cpool.tile([P, CH], BF16, tag="warm")
            nc.gpsimd.memset(warm[:], 0.0)
            # warm-up exp so the ACT table set loads during the DMA ramp
            warm2 = cpool.tile([HP, 1], F32, tag="warm2")
            nc.scalar.activation(
                warm2[:], ident[:, 0:1], mybir.ActivationFunctionType.Exp
            )

            # persistent tiles
            xt_sb = [[None] * NCH for _ in range(ET)]
            qt_all = qkpool.tile([H, T], BF16, tag="qt")
            kt_all = qkpool.tile([H, T], BF16, tag="kt")
            vt_all = qkpool.tile([H, T], BF16, tag="vt")
            # v (ones-cols set once at startup)
            v_all = qkpool.tile([P, NIT * HP], BF16, tag="v_all")
            v_3d = v_all[:].rearrange("p (n f) -> p n f", f=HP)
            nc.gpsimd.memset(v_3d[:, :, H:HP], 1.0)

            wqk_all = cpool.tile([P, ET * 2 * H], BF16, tag="wqk")
            nc.scalar.dma_start(out=wqk_all[:], in_=wqk_d[:, :])
            wqk_sb = [wqk_all[:, e * 2 * H : (e + 1) * 2 * H] for e in range(ET)]
            nb_sb = cpool.tile([1, 1], mybir.dt.uint32, tag="nb")
            nc.gpsimd.dma_start(out=nb_sb[:], in_=nb_d[:, :])

            dma_engines = [nc.sync, nc.scalar, nc.gpsimd]

            def emit_xt_dmas(c):
                for e in range(ET):
                    xt = xtpool.tile([P, CH], BF16, tag=f"xt{e}_{c}")
                    # chunks 0-1 split over 3 queues (fast ramp; scalar and
                    # gpsimd queues are idle until attention starts); the
                    # rest on sync only so those queues stay clean
                    eng = dma_engines[e % 3] if c <= 1 else nc.sync
                    eng.dma_start(
                        out=xt[:],
                        in_=xt_d[(c * ET + e) * P : (c * ET + e + 1) * P, :],
                    )
                    xt_sb[e][c] = xt

            emit_xt_dmas(0)
            wv_all = cpool.tile([P, ET * H], BF16, tag="wv")
            nc.sync.dma_start(out=wv_all[:], in_=wv_d[:, :])
            wv_sb = [wv_all[:, e * H : (e + 1) * H] for e in range(ET)]
            m_sb = cpool.tile([P, NIT], F32, tag="m")
            nc.sync.dma_start(out=m_sb[:], in_=m_d[:, :])
            im_sb = cpool.tile([P, NIT], F32, tag="im")
            nc.sync.dma_start(out=im_sb[:], in_=im_d[:, :])
            for c in range(1, NCH):
                emit_xt_dmas(c)

            # PE warm-ups: release the HAM clock gate during the DMA ramp
            for i in range(NWARM):
                pwarm = pp.tile([P, CH], F32, tag="pp")
                nc.tensor.matmul(
                    pwarm[:], warm[:, 0:P], warm[:], start=True, stop=True
                )

            def emit_proj_qk(c):
                # q/k projection (fused): psum[0:64]=qT, [64:128]=kT
                pqk = pp.tile([P, CH], F32, tag="pp")
                for e in range(ET):
                    nc.tensor.matmul(
                        pqk[:],
                        wqk_sb[e],
                        xt_sb[e][c][:],
                        start=(e == 0),
                        stop=(e == ET - 1),
                    )
                nc.vector.tensor_copy(
                    qt_all[:, c * CH : (c + 1) * CH], pqk[0:H, :]
                )
                nc.vector.tensor_copy(
                    kt_all[:, c * CH : (c + 1) * CH], pqk[H : 2 * H, :]
                )

            def emit_proj_v(c):
                # v projection (vT)
                pv = pp.tile([H, CH], F32, tag="pp")
                for e in range(ET):
                    nc.tensor.matmul(
                        pv[:],
                        wv_sb[e],
                        xt_sb[e][c][:],
                        start=(e == 0),
                        stop=(e == ET - 1),
                    )
                nc.vector.tensor_copy(vt_all[:, c * CH : (c + 1) * CH], pv[:])
                # transpose the 4 128-blocks into one psum tile, then one
                # strided DVE copy into v_all
                pvt4 = ps_t.tile([P, 4 * H], F32, tag="pst")
                for k in range(4):
                    it = c * 4 + k
                    nc.tensor.matmul(
                        pvt4[:, k * H : (k + 1) * H],
                        vt_all[:, it * P : (it + 1) * P],
                        ident_b[0:H, 0:H],
                        start=True,
                        stop=True,
                    )
                nc.vector.tensor_copy(
                    v_3d[:, c * 4 : (c + 1) * 4, 0:H],
                    pvt4[:].rearrange("p (n h) -> p n h", h=H),
                )

            def emit_attn(c):
                # scores^T + exp + PV over j-tile PAIRS (one 2-bank psum
                # tile + one wide exp per pair), software-pipelined
                po = ps_po.tile([HP, CH], F32, tag="po")
                njt = 4 * c + 4

                def emit_pv(pt, segs):
                    for j, off, w, base in segs:
                        nc.tensor.matmul(
                            po[:, off:CH],
                            v_all[:, j * HP : (j + 1) * HP],
                            pt[:, base : base + w],
                            start=(j == 0),
                            stop=(j == njt - 1),
                        )

                pend = None
                for p in range(njt // 2):
                    j0, j1 = 2 * p, 2 * p + 1
                    off0 = max(0, j0 * P - c * CH)
                    off1 = max(0, j1 * P - c * CH)
                    w0, w1 = CH - off0, CH - off1
                    # j1's scores at fixed base CH so neither matmul's psum
                    # region straddles a bank boundary
                    pw = CH + w1
                    pss = ps_s.tile([P, pw], F32, tag="pss")
                    nc.tensor.matmul(
                        pss[:, 0:w0],
                        kt_all[:, j0 * P : (j0 + 1) * P],
                        qt_all[:, c * CH + off0 : (c + 1) * CH],
                        start=True,
                        stop=True,
                    )
                    nc.tensor.matmul(
                        pss[:, CH : CH + w1],
                        kt_all[:, j1 * P : (j1 + 1) * P],
                        qt_all[:, c * CH + off1 : (c + 1) * CH],
                        start=True,
                        stop=True,
                    )
                    pt = ptpool.tile([P, pw], BF16, tag="pt")
                    # one wide exp per pair (cols [w0:CH] are unused garbage
                    # when w0 < CH; never read downstream)
                    nc.scalar.activation(
                        pt[:], pss[:], mybir.ActivationFunctionType.Exp,
                        scale=SCALE,
                    )
                    for j, base in ((j0, 0), (j1, CH)):
                        if j >= 4 * c:
                            # diag block (segment cols 0:128): zero keys
                            # below the diagonal: keep where (col-row) >= 0
                            nc.gpsimd.affine_select(
                                out=pt[:, base : base + P],
                                in_=pt[:, base : base + P],
                                compare_op=mybir.AluOpType.is_ge,
                                fill=0.0,
                                base=0,
                                channel_multiplier=-1,
                                pattern=[[1, P]],
                            )
                    if pend is not None:
                        emit_pv(*pend)
                    pend = (pt, ((j0, off0, w0, 0), (j1, off1, w1, CH)))
                emit_pv(*pend)
                return po

            def emit_blend_pre(po):
                # free the po psum slot early: one DVE cast to sbuf bf16
                po_b = blpool.tile([HP, CH], BF16, tag="pob")
                nc.vector.tensor_copy(po_b[:], po[:])
                return po_b

            def _blend_full(c, po_b, ob):
                # per-128-block transpose -> [i,h], normalize by 1/denom
                # (col 64), blend rows >= len with v
                pot4 = ps_t.tile([P, 4 * HP], F32, tag="pst")
                for k in range(4):
                    nc.tensor.matmul(
                        pot4[:, k * HP : (k + 1) * HP],
                        po_b[:, k * P : (k + 1) * P],
                        ident_b[:],
                        start=True,
                        stop=True,
                    )
                # batched reciprocal of the 4 denominators (strided AP),
                # then masked: rm4 = m / denom
                pot3d = pot4[:].rearrange("p (k f) -> p k f", f=HP)
                den4 = blpool.tile([P, 4], F32, tag="den4")
                nc.vector.tensor_copy(
                    den4[:].rearrange("p (k f) -> p k f", f=1),
                    pot3d[:, :, H : H + 1],
                )
                recip4 = blpool.tile([P, 4], F32, tag="recip4")
                nc.vector.reciprocal(recip4[:], den4[:])
                rm4 = blpool.tile([P, 4], F32, tag="rm4")
                nc.vector.tensor_mul(
                    rm4[:], recip4[:], m_sb[:, c * 4 : (c + 1) * 4]
                )
                for k in range(4):
                    it = c * 4 + k
                    pk = pot4[:, k * HP : k * HP + H]
                    t1 = blpool.tile([P, H], F32, tag="t1")
                    nc.vector.tensor_scalar_mul(t1[:], pk, rm4[:, k : k + 1])
                    # ob = (v * im) + t1   (one fused op)
                    nc.vector.scalar_tensor_tensor(
                        ob[:, k * H : (k + 1) * H],
                        v_3d[:, it, 0:H],
                        im_sb[:, it : it + 1],
                        t1[:],
                        op0=mybir.AluOpType.mult,
                        op1=mybir.AluOpType.add,
                    )

            def emit_blend(c, po_b):
                ob = obpool.tile([P, 4 * H], F32, tag="ob")
                if c < NCH - 1:
                    _blend_full(c, po_b, ob)
                else:
                    # skipped quarters: out rows are just v
                    with tc.If(nbv > 4 * c) as cmp:
                        _blend_full(c, po_b, ob)
                    with cmp.Else():
                        nc.vector.tensor_copy(
                            ob[:].rearrange("p (n h) -> p n h", h=H),
                            v_3d[:, c * 4 : (c + 1) * 4, 0:H],
                        )
                nc.sync.dma_start(
                    out=out_d[:, c * 4 * H : (c + 1) * 4 * H],
                    in_=ob[:],
                )

            # main schedule: free po (cast), proj chunk c, blend of quarter
            # c-1 (PE transposes slot in between), attention quarter c.
            # Quarters/projections beyond the valid length (nbv i-tiles) are
            # skipped at runtime; the unguarded blend is self-correcting for
            # skipped quarters (m=0 rows, stale-but-positive denominators).
            po_prev = None
            for c in range(NCH):
                po_b = emit_blend_pre(po_prev) if po_prev is not None else None
                if c < NCH - 1:
                    emit_proj_qk(c)
                    emit_proj_v(c)
                    if po_b is not None:
                        emit_blend(c - 1, po_b)
                    po_prev = emit_attn(c)
                else:
                    # last quarter: v-proj first (always needed), then the
                    # previous blend (starts its DVE chain early), then q/k
                    # proj + attention inside ONE runtime guard — both are
                    # dead work when the valid length ends before this
                    # quarter (each extra If-merge costs a cross-engine sync).
                    # The nb register loads (~1.2us TENSOR_LOAD per engine)
                    # are emitted only here, NOT during the startup ramp.
                    guard_engines = [
                        mybir.EngineType.PE,
                        mybir.EngineType.Activation,
                        mybir.EngineType.DVE,
                        mybir.EngineType.Pool,
                    ]
                    nb_regs = bass.RegisterHandles(
                        nc.alloc_register(e, f"nb_{e.name}")
                        for e in guard_engines
                    )
                    for e in guard_engines:
                        nc.engines[e].reg_load(nb_regs[e], nb_sb[0:1, 0:1])
                    nbv = nc.snap(
                        nb_regs, donate=True, min_val=1, max_val=NIT
                    )
                    emit_proj_v(c)
                    if po_b is not None:
                        emit_blend(c - 1, po_b)
                    with tc.If(nbv > 4 * c):
                        emit_proj_qk(c)
                        po_prev = emit_attn(c)
            po_b = emit_blend_pre(po_prev)
            emit_blend(NCH - 1, po_b)

    nc.compile()
    return nc


_NC_CACHE = None


def _get_nc():
    global _NC_CACHE
    if _NC_CACHE is None:
        _NC_CACHE = build_nc()
    return _NC_CACHE


def make_in_maps(x, Wq, Wk, Wv, lengths):
    bf16 = mybir.dt.np(BF16)
    wqk_f = np.concatenate(
        [np.asarray(Wq, dtype=np.float32), np.asarray(Wk, dtype=np.float32)],
        axis=1,
    )  # [E, 128]
    # pack p-major: [128, e*128] with wqk_p[p, e*F+f] = wqk_f[e*128+p, f]
    wqk = np.ascontiguousarray(
        wqk_f.reshape(ET, P, 2 * H).transpose(1, 0, 2).reshape(P, ET * 2 * H)
    ).astype(bf16)
    wv_f = np.asarray(Wv, dtype=np.float32)
    wv = np.ascontiguousarray(
        wv_f.reshape(ET, P, H).transpose(1, 0, 2).reshape(P, ET * H)
    ).astype(bf16)
    in_maps = []
    for b in range(B):
        xtb = np.asarray(x[b], dtype=np.float32).T  # [E, T]
        # repack chunk-major: block (c, e) contiguous [128, 512]
        xt = np.ascontiguousarray(
            xtb.reshape(ET, P, NCH, CH).transpose(2, 0, 1, 3).reshape(
                NCH * ET * P, CH
            )
        ).astype(bf16)
        mflat = (np.arange(T) < int(lengths[b])).astype(np.float32)
        m = np.ascontiguousarray(mflat.reshape(NIT, P).T)  # [128, 16]
        im = np.ascontiguousarray(1.0 - m)
        nb = np.array(
            [[(int(lengths[b]) + P - 1) // P]], dtype=np.uint32
        )
        in_maps.append(
            {"xt": xt, "wqk": wqk, "wv": wv, "m": m, "im": im, "nb": nb}
        )
    return in_maps


def run(x, Wq, Wk, Wv, lengths, trace=False):
    nc = _get_nc()
    in_maps = make_in_maps(x, Wq, Wk, Wv, lengths)
    res = run_bass_kernel_spmd(
        nc, in_maps, core_ids=list(range(B)), trace=trace
    )
    # out is p-major [128, 16*64] -> [T, H]
    out = np.stack(
        [
            np.ascontiguousarray(
                res.results[b]["out"].reshape(P, NIT, H).transpose(1, 0, 2)
            ).reshape(T, H)
            for b in range(B)
        ],
        axis=0,
    )
    return out, res


def kernel(x, Wq, Wk, Wv, lengths):
    out, _ = run(x, Wq, Wk, Wv, lengths, trace=False)
    return out
